# revision 8
# baseline (speedup 1.0000x reference)
"""Trainium2 Bass kernel for nn_Decoder (Bahdanau attention + GRU step + vocab projection).

Distribution over 8 NeuronCores:
  - Attention: data-parallel over batch (8 rows/core), processed in two
    512-row halves so softmax/context of half 0 overlap matmuls of half 1.
  - gin = [context; emb[x]] (transposed, bf16) is all-gathered on device;
    every core then computes the full-batch GRU step redundantly (cheap, and
    it keeps the PE warm between the collective and the projection).
  - fc projection [1024,32000]: tensor-parallel vocab shard of 4000/core.
Host side: shards/transposes inputs, casts weights per dtype config,
assembles full outputs from per-core shards.
"""
from contextlib import ExitStack

import numpy as np
import ml_dtypes

import concourse.bass as bass
import concourse.mybir as mybir
import concourse.tile as tile
from concourse import bacc
from concourse.bass_utils import run_bass_kernel_spmd
from concourse.masks import make_identity

F32 = mybir.dt.float32
F32R = mybir.dt.float32r
BF16 = mybir.dt.bfloat16
I32 = mybir.dt.int32
AF = mybir.ActivationFunctionType
ALU = mybir.AluOpType
AX = mybir.AxisListType

N_CORES = 8
B, T, U, E, VOCAB = 64, 128, 1024, 256, 32000
BL = B // N_CORES          # 8 batch rows per core
R = BL * T                 # 1024 flattened rows per core
KG = U + E                 # 1280 GRU input dim
G2 = 2 * U                 # 2048 used gate columns (z, hh)
VS = VOCAB // N_CORES      # 4000 vocab columns per core
NK = U // 128              # 8 u_in chunks
NM = U // 128              # 8 u_out chunks
NKG = KG // 128            # 10 gru contraction chunks
NVN = 8                    # fc vocab N-chunks
VN = VS // NVN             # 500
HB = 4                     # batches per half

# dtype per matmul stage: "bf16" or "f32r" (f32 data, relaxed-precision matmul)
CFG = {
    "mm1": "bf16",     # enc @ W1 (+ W2 path dtype)
    "scores": "bf16",  # tanh(S) @ V
    "gru": "bf16",     # gin @ gru_k (also the all-gather payload dtype)
    "fc": "bf16",      # state @ fc_W
}


def _dt(stage):
    return BF16 if CFG[stage] == "bf16" else F32


def _np_dt(stage):
    return ml_dtypes.bfloat16 if CFG[stage] == "bf16" else np.float32


def _mm(ap, stage):
    """Bitcast matmul operand APs to float32r for relaxed fp32 matmuls."""
    if CFG[stage] == "f32r":
        return ap.bitcast(F32R)
    return ap


def build_nc():
    nc = bacc.Bacc("TRN2", target_bir_lowering=False, debug=False,
                   num_devices=N_CORES)

    d_mm1, d_sc, d_gru, d_fc = (_dt(s) for s in ("mm1", "scores", "gru", "fc"))

    # ---- kernel I/O ----
    enc_t = nc.dram_tensor("enc_t", [U, R], d_mm1, kind="ExternalInput").ap()
    w1 = nc.dram_tensor("w1", [U, U], d_mm1, kind="ExternalInput").ap()
    w2 = nc.dram_tensor("w2", [U, U], d_mm1, kind="ExternalInput").ap()
    hidden_t = nc.dram_tensor("hidden_t", [U, BL], d_mm1, kind="ExternalInput").ap()
    b1t = nc.dram_tensor("b1t", [128, NM], F32, kind="ExternalInput").ap()
    b2t = nc.dram_tensor("b2t", [128, NM], F32, kind="ExternalInput").ap()
    v_t = nc.dram_tensor("v_t", [128, NM], d_sc, kind="ExternalInput").ap()
    emb = nc.dram_tensor("emb", [VOCAB, E], F32, kind="ExternalInput").ap()
    x_idx = nc.dram_tensor("x_idx", [BL, 1], I32, kind="ExternalInput").ap()
    gru_k = nc.dram_tensor("gru_k_zh", [KG, G2], d_gru, kind="ExternalInput").ap()
    gru_b = nc.dram_tensor("gru_b_zh", [1, G2], F32, kind="ExternalInput").ap()
    fc_w = nc.dram_tensor("fc_w", [U, VS], d_fc, kind="ExternalInput").ap()
    fc_b = nc.dram_tensor("fc_b", [1, VS], d_fc, kind="ExternalInput").ap()

    out_logits = nc.dram_tensor("out_logits", [B, VS], F32, kind="ExternalOutput").ap()
    out_state = nc.dram_tensor("out_state", [B, U], F32, kind="ExternalOutput").ap()
    out_attn = nc.dram_tensor("out_attn", [BL, T], F32, kind="ExternalOutput").ap()

    def bcast(row_ap, parts):
        """[1, N] (DRAM) access pattern -> [parts, N] partition-broadcast."""
        inner = [list(d) for d in row_ap.ap if d[1] != 1]
        return bass.AP(tensor=row_ap.tensor, offset=row_ap.offset,
                       ap=[[0, parts]] + inner)

    with tile.TileContext(nc) as tc, ExitStack() as es:
        consts = es.enter_context(tc.tile_pool(name="consts", bufs=1))
        enc_p = es.enter_context(tc.tile_pool(name="enc", bufs=NK))
        w1_p = es.enter_context(tc.tile_pool(name="w1", bufs=NK))
        w2_p = es.enter_context(tc.tile_pool(name="w2", bufs=3))
        tanh_p = es.enter_context(tc.tile_pool(name="tanh", bufs=NM))
        fcw_p = es.enter_context(tc.tile_pool(name="fcw", bufs=8))
        gruk_p = es.enter_context(tc.tile_pool(name="gruk", bufs=3))
        small = es.enter_context(tc.tile_pool(name="small", bufs=1))
        abc_p = es.enter_context(tc.tile_pool(name="abc", bufs=BL))
        psum = es.enter_context(tc.tile_pool(name="psum", bufs=8, space="PSUM"))
        dram = es.enter_context(tc.tile_pool(name="dram", bufs=4, space="DRAM"))

        # ============ phase 0a: critical-path DMAs + constants ============
        hid_sb = consts.tile([128, NK, BL], d_mm1)
        nc.sync.dma_start(out=hid_sb[:],
                          in_=hidden_t.rearrange("(k p) b -> p k b", p=128))
        enc_sb, w1_sb = [], []
        for k in range(NK):
            te = enc_p.tile([128, R], d_mm1, tag="enc", name=f"enc{k}")
            nc.sync.dma_start(out=te[:], in_=enc_t[k * 128:(k + 1) * 128, :])
            enc_sb.append(te)
            tw = w1_p.tile([128, U], d_mm1, tag="w1", name=f"w1_{k}")
            nc.sync.dma_start(out=tw[:], in_=w1[k * 128:(k + 1) * 128, :])
            w1_sb.append(tw)

        ident = consts.tile([128, 128], F32)
        make_identity(nc, ident[:])
        ones = consts.tile([1, B], d_fc)
        nc.vector.memset(ones[:], 1.0)
        v_sb = consts.tile([128, NM], d_sc)
        nc.sync.dma_start(out=v_sb[:], in_=v_t[:])
        b1_sb = consts.tile([128, NM], F32)
        nc.sync.dma_start(out=b1_sb[:], in_=b1t[:])
        b2_sb = consts.tile([128, NM], F32)
        nc.sync.dma_start(out=b2_sb[:], in_=b2t[:])
        b12_sb = consts.tile([128, NM], F32)
        nc.vector.tensor_add(out=b12_sb[:], in0=b1_sb[:], in1=b2_sb[:])

        # ============ phase 0b: h2 = hidden @ W2, transposed + biases ======
        h2_sb = small.tile([BL, U], F32, tag="h2nat")
        ph = [psum.tile([BL, 512], F32, tag="pb", name=f"ph{n}") for n in range(2)]
        for k in range(NK):
            w2_t = w2_p.tile([128, U], d_mm1, tag="w2", name=f"w2t{k}")
            nc.sync.dma_start(out=w2_t[:], in_=w2[k * 128:(k + 1) * 128, :])
            for n in range(2):
                nc.tensor.matmul(
                    _mm(ph[n][:], "mm1"),
                    _mm(hid_sb[:, k, :], "mm1"),
                    _mm(w2_t[:, n * 512:(n + 1) * 512], "mm1"),
                    start=(k == 0), stop=(k == NK - 1))
        for n in range(2):
            nc.vector.tensor_copy(out=h2_sb[:, n * 512:(n + 1) * 512], in_=ph[n][:])

        h2t_sb = consts.tile([128, NM, BL], F32)
        for m in range(NM):
            pt = psum.tile([128, BL], F32, tag="pb", name=f"h2t_ps{m}")
            nc.tensor.transpose(pt[:], h2_sb[:, m * 128:(m + 1) * 128],
                                ident[:BL, :BL])
            nc.vector.tensor_scalar(
                out=h2t_sb[:, m, :], in0=pt[:],
                scalar1=b12_sb[:, m:m + 1], scalar2=None, op0=ALU.add)

        # ============ phase 0c: embedding gather -> ginT chunks 8..9 ======
        idx_sb = small.tile([BL, 1], I32, tag="idx")
        nc.sync.dma_start(out=idx_sb[:], in_=x_idx[:])
        xe_sb = small.tile([BL, E], F32, tag="xe")
        nc.gpsimd.indirect_dma_start(
            out=xe_sb[:], out_offset=None, in_=emb[:],
            in_offset=bass.IndirectOffsetOnAxis(ap=idx_sb[:, :1], axis=0))
        gin_sb = [small.tile([128, BL], d_gru, tag=f"gin{k}", name=f"gin_sb{k}")
                  for k in range(NKG)]
        for j in range(2):
            pt = psum.tile([128, BL], F32, tag="pb", name=f"xe_ps{j}")
            nc.tensor.transpose(pt[:], xe_sb[:, j * 128:(j + 1) * 128],
                                ident[:BL, :BL])
            nc.vector.tensor_copy(out=gin_sb[NK + j][:], in_=pt[:])

        # ============ phase 1: per-half attention pipeline =================
        # rows are batch-major: half h covers batches 4h..4h+3 entirely, so
        # softmax/context for half 0 run while the PE works on half 1.
        tanh_sb = [tanh_p.tile([128, R], d_sc, tag="tanh", name=f"tanh_sb{m}")
                   for m in range(NM)]
        sc_dram = dram.tile([1, R], F32)
        attn_dram = dram.tile([BL, T], F32)
        for h in range(2):
            rows = slice(h * 512, (h + 1) * 512)
            for m in range(NM):
                ps = psum.tile([128, 512], F32, tag="pb", name=f"mm1_{h}_{m}")
                for k in range(NK):
                    nc.tensor.matmul(
                        _mm(ps[:], "mm1"),
                        _mm(w1_sb[k][:, m * 128:(m + 1) * 128], "mm1"),
                        _mm(enc_sb[k][:, rows], "mm1"),
                        start=(k == 0), stop=(k == NK - 1))
                for b in range(HB):
                    gb = h * HB + b
                    nc.scalar.activation(
                        out=tanh_sb[m][:, gb * T:(gb + 1) * T],
                        in_=ps[:, b * T:(b + 1) * T],
                        func=AF.Tanh, bias=h2t_sb[:, m, gb:gb + 1], scale=1.0)
            psc = psum.tile([1, 512], F32, tag="pb", name=f"sc_ps{h}")
            for m in range(NM):
                nc.tensor.matmul(
                    _mm(psc[:], "scores"),
                    _mm(v_sb[:, m:m + 1], "scores"),
                    _mm(tanh_sb[m][:, rows], "scores"),
                    start=(m == 0), stop=(m == NM - 1))
            sc_sb = small.tile([1, 512], F32, tag="scsb", bufs=2,
                               name=f"sc_sb{h}")
            nc.vector.tensor_copy(out=sc_sb[:], in_=psc[:])
            nc.sync.dma_start(out=sc_dram[:, rows], in_=sc_sb[:])

            # softmax for the 4 batches of this half
            sm_sb = small.tile([HB, T], F32, tag="smx", bufs=2, name=f"sm{h}")
            nc.sync.dma_start(
                out=sm_sb[:],
                in_=sc_dram[0:1, rows].rearrange("o (b t) -> (o b) t", t=T))
            nmx = small.tile([HB, 1], F32, tag="nmx", bufs=2, name=f"nmx{h}")
            nc.vector.tensor_reduce(out=nmx[:], in_=sm_sb[:], axis=AX.X,
                                    op=ALU.max, negate=True)
            ex_sb = small.tile([HB, T], F32, tag="ex", bufs=2, name=f"ex{h}")
            esum = small.tile([HB, 1], F32, tag="esum", bufs=2, name=f"esum{h}")
            nc.scalar.activation(out=ex_sb[:], in_=sm_sb[:], func=AF.Exp,
                                 bias=nmx[:, :1], scale=1.0,
                                 accum_out=esum[:, :1])
            rsum = small.tile([HB, 1], F32, tag="rsum", bufs=2, name=f"rsum{h}")
            nc.vector.reciprocal(out=rsum[:], in_=esum[:])
            attn_sb = small.tile([HB, T], F32, tag="attn", bufs=2,
                                 name=f"attn{h}")
            nc.vector.tensor_scalar_mul(out=attn_sb[:], in0=ex_sb[:],
                                        scalar1=rsum[:, :1])
            nc.sync.dma_start(out=out_attn[h * HB:(h + 1) * HB, :], in_=attn_sb[:])
            nc.sync.dma_start(out=attn_dram[h * HB:(h + 1) * HB, :], in_=attn_sb[:])

            # context for these 4 batches (DVE), accumulating ginT chunks
            for b in range(HB):
                gb = h * HB + b
                t = abc_p.tile([128, T], F32, tag="abc", name=f"abc{h}_{b}")
                nc.gpsimd.dma_start(out=t[:],
                                    in_=bcast(attn_dram[gb:gb + 1, :], 128))
                scratch = small.tile([128, T], F32, tag="ttr_scratch", bufs=2,
                                     name=f"scr{h}_{b}")
                ctxf = small.tile([128, NK], F32, tag="ctxf", bufs=2,
                                  name=f"ctxf{h}_{b}")
                for k in range(NK):
                    nc.vector.scalar_tensor_tensor(
                        out=scratch[:],
                        in0=enc_sb[k][:, gb * T:(gb + 1) * T],
                        scalar=1.0, in1=t[:],
                        op0=ALU.mult, op1=ALU.mult,
                        accum_out=ctxf[:, k:k + 1])
                for k in range(NK):
                    nc.vector.tensor_copy(out=gin_sb[k][:, b + h * HB:b + h * HB + 1],
                                          in_=ctxf[:, k:k + 1])

        # ============ phase 2: all-gather ginT across the 8 cores =========
        cc_in = dram.tile([KG, BL], d_gru)
        st_gin = small.tile([128, NKG, BL], d_gru, tag="stgin")
        for k in range(NKG):
            nc.vector.tensor_copy(out=st_gin[:, k, :], in_=gin_sb[k][:])
        nc.sync.dma_start(out=cc_in[:].rearrange("(k p) b -> p k b", p=128),
                          in_=st_gin[:])
        cc_out = dram.tile([N_CORES, KG, BL], d_gru)
        nc.gpsimd.collective_compute(
            "AllGather", ALU.bypass,
            replica_groups=[list(range(N_CORES))],
            ins=[cc_in.opt()], outs=[cc_out.opt()])

        gfull = []
        for k in range(NKG):
            gf = small.tile([128, B], d_gru, tag="gfull", bufs=NKG,
                            name=f"gfull{k}")
            nc.sync.dma_start(
                out=gf[:].rearrange("p (r b) -> p r b", r=N_CORES),
                in_=cc_out[:, k * 128:(k + 1) * 128, :].rearrange(
                    "r p b -> p r b"))
            gfull.append(gf)

        # ============ phase 3: full-batch GRU step (replicated) ===========
        pg = [psum.tile([B, 512], F32, tag="pb", name=f"pg{n}")
              for n in range(4)]
        for k in range(NKG):
            gt = gruk_p.tile([128, G2], d_gru, tag="gruk", name=f"gruk{k}")
            nc.sync.dma_start(out=gt[:], in_=gru_k[k * 128:(k + 1) * 128, :])
            for n in range(4):
                nc.tensor.matmul(
                    _mm(pg[n][:], "gru"),
                    _mm(gfull[k][:], "gru"),
                    _mm(gt[:, n * 512:(n + 1) * 512], "gru"),
                    start=(k == 0), stop=(k == NKG - 1))
        z_sb = small.tile([B, U], F32, tag="z")
        hh_sb = small.tile([B, U], F32, tag="hh")
        for n in range(4):
            gb_bc = small.tile([B, 512], F32, tag="gbbc", bufs=2,
                               name=f"gbbc{n}")
            nc.gpsimd.dma_start(
                out=gb_bc[:], in_=bcast(gru_b[0:1, n * 512:(n + 1) * 512], B))
            gsum = small.tile([B, 512], F32, tag="gsum", bufs=2,
                              name=f"gsum{n}")
            nc.vector.tensor_add(out=gsum[:], in0=pg[n][:], in1=gb_bc[:])
            if n < 2:
                nc.scalar.activation(out=z_sb[:, n * 512:(n + 1) * 512],
                                     in_=gsum[:], func=AF.Sigmoid)
            else:
                nc.scalar.activation(out=hh_sb[:, (n - 2) * 512:(n - 1) * 512],
                                     in_=gsum[:], func=AF.Tanh)
        state_sb = small.tile([B, U], F32, tag="state")
        zh_sb = small.tile([B, U], F32, tag="zh")
        nc.vector.tensor_mul(out=zh_sb[:], in0=z_sb[:], in1=hh_sb[:])
        nc.vector.tensor_sub(out=state_sb[:], in0=hh_sb[:], in1=zh_sb[:])
        nc.sync.dma_start(out=out_state[:], in_=state_sb[:])

        # transpose state -> statT [u, batch] (d_fc) for the projection
        stT = []
        for m in range(NM):
            pt = psum.tile([128, B], F32, tag="pb", name=f"st_ps{m}")
            nc.tensor.transpose(pt[:], state_sb[:, m * 128:(m + 1) * 128],
                                ident[:B, :B])
            sb_ = small.tile([128, B], d_fc, tag="sTb", bufs=NM,
                             name=f"stT{m}")
            nc.vector.tensor_copy(out=sb_[:], in_=pt[:])
            stT.append(sb_)

        # ============ phase 4: fc projection (vocab shard) ================
        fcw_sb = []
        for k in range(NK):
            t = fcw_p.tile([128, VS], d_fc, tag="fcw", name=f"fcw{k}")
            nc.sync.dma_start(out=t[:], in_=fc_w[k * 128:(k + 1) * 128, :])
            fcw_sb.append(t)
        fcb_sb = small.tile([1, VS], d_fc, tag="fcb")
        nc.sync.dma_start(out=fcb_sb[:], in_=fc_b[:])

        pl = [psum.tile([B, VN], F32, tag="pb", name=f"pl{n}")
              for n in range(NVN)]
        for k in range(NK):
            for n in range(NVN):
                nc.tensor.matmul(
                    _mm(pl[n][:], "fc"),
                    _mm(stT[k][:], "fc"),
                    _mm(fcw_sb[k][:, n * VN:(n + 1) * VN], "fc"),
                    start=(k == 0), stop=False)
        for n in range(NVN):
            nc.tensor.matmul(
                _mm(pl[n][:], "fc"),
                _mm(ones[:], "fc"),
                _mm(fcb_sb[:, n * VN:(n + 1) * VN], "fc"),
                start=False, stop=True)
            lo = small.tile([B, VN], F32, tag="lo", bufs=3, name=f"lo{n}")
            nc.vector.tensor_copy(out=lo[:], in_=pl[n][:])
            nc.sync.dma_start(out=out_logits[:, n * VN:(n + 1) * VN], in_=lo[:])

    nc.compile()
    return nc


def shard_inputs(x, hidden, enc_output, emb, W1, b1, W2, b2, V, bV,
                 gru_k, gru_rk, gru_b, fc_W, fc_b):
    """Build the 8 per-core input maps from full inputs (host-side prep)."""
    f32 = np.float32
    d_mm1, d_sc, d_gru, d_fc = (_np_dt(s) for s in ("mm1", "scores", "gru", "fc"))

    x = np.asarray(x).astype(np.int32).reshape(B, 1)
    hidden = np.asarray(hidden, f32)
    enc_output = np.asarray(enc_output, f32)
    emb_np = np.ascontiguousarray(np.asarray(emb, f32))
    W1_np = np.ascontiguousarray(np.asarray(W1, f32).astype(d_mm1))
    W2_np = np.ascontiguousarray(np.asarray(W2, f32).astype(d_mm1))
    b1t = np.ascontiguousarray(np.asarray(b1, f32).reshape(NM, 128).T)
    b2t = np.ascontiguousarray(np.asarray(b2, f32).reshape(NM, 128).T)
    v_t = np.ascontiguousarray(np.asarray(V, f32)[:, 0].reshape(NM, 128).T
                               .astype(d_sc))
    gk = np.asarray(gru_k, f32)
    gru_k_zh = np.ascontiguousarray(
        np.concatenate([gk[:, :U], gk[:, 2 * U:]], axis=1).astype(d_gru))
    gb = np.asarray(gru_b, f32)
    gru_b_zh = np.ascontiguousarray(
        np.concatenate([gb[:U], gb[2 * U:]])[None, :].astype(f32))
    fc_W_np = np.asarray(fc_W, f32)
    fc_b_np = np.asarray(fc_b, f32)

    in_maps = []
    for c in range(N_CORES):
        sl = slice(c * BL, (c + 1) * BL)
        enc_c = enc_output[sl].reshape(R, U)
        in_maps.append({
            "enc_t": np.ascontiguousarray(enc_c.T).astype(d_mm1),
            "w1": W1_np,
            "w2": W2_np,
            "hidden_t": np.ascontiguousarray(hidden[sl].T).astype(d_mm1),
            "b1t": b1t,
            "b2t": b2t,
            "v_t": v_t,
            "emb": emb_np,
            "x_idx": np.ascontiguousarray(x[sl]),
            "gru_k_zh": gru_k_zh,
            "gru_b_zh": gru_b_zh,
            "fc_w": np.ascontiguousarray(
                fc_W_np[:, c * VS:(c + 1) * VS]).astype(d_fc),
            "fc_b": np.ascontiguousarray(
                fc_b_np[c * VS:(c + 1) * VS][None, :]).astype(d_fc),
        })
    return in_maps


def assemble(results):
    logits = np.concatenate([results[c]["out_logits"] for c in range(N_CORES)],
                            axis=1).astype(np.float32)
    state = np.asarray(results[0]["out_state"], np.float32)
    attn = np.concatenate([results[c]["out_attn"] for c in range(N_CORES)],
                          axis=0).astype(np.float32)[..., None]
    return logits, state, attn


_NC_CACHE = {}


def kernel(**inputs):
    key = tuple(sorted(CFG.items()))
    if key not in _NC_CACHE:
        _NC_CACHE[key] = build_nc()
    nc = _NC_CACHE[key]
    in_maps = shard_inputs(**inputs)
    res = run_bass_kernel_spmd(nc, in_maps, list(range(N_CORES)))
    return assemble(res.results)
